# revision 20
# baseline (speedup 1.0000x reference)
"""Trainium2 Bass kernel: pre-norm decoder block (B=2, S=2048, D=1024, H=16, DFF=4096).

Sharding: 8 cores = 2 data-parallel groups (one per batch) x 4 tensor-parallel
ranks. Attention is head-sharded (4 heads/core, Megatron column-parallel QKV).
Each rank computes its partial Wo contribution (row-parallel Wo) per 512-wide
sequence block as attention for that block completes; a single bf16
ReduceScatter over the sequence axis then hands every rank the fully-reduced
pre-residual h for its own 512 rows. The rest (residual + LN2 + FFN +
residual) runs sequence-sharded with full w1/w2 (no further collectives).

Everything on-chip is bf16 (psum accumulation fp32). Elementwise work is
batched into as few, as wide instructions as possible (paired-psum tiles with
one activation/copy over both halves) since per-instruction overhead
dominates on this target. LayerNorm statistics are computed by DVE
pairwise-reduction trees plus ones-vector matmuls; the per-position LN affine
is folded into the projections via extra contraction rows (LN1) or a
broadcasted scale/shift (LN2).
"""

import numpy as np
import ml_dtypes

import concourse.bass as bass
import concourse.mybir as mybir
import concourse.tile as tile
from concourse import bacc
from concourse.bass_utils import run_bass_kernel_spmd

BF = mybir.dt.bfloat16
F8 = mybir.dt.float8e4
F32 = mybir.dt.float32
AF = mybir.ActivationFunctionType
ALU = mybir.AluOpType

B, S, D, H, DFF = 2, 2048, 1024, 16, 4096
DH = D // H
EPS = 1e-5

N_CORES = 8
TP = 4                    # tensor-parallel ranks per group
HC = H // TP              # heads per core
DC = HC * DH              # head features per core
RC = S // TP              # seq rows per core in stage B
FT = D // 128             # feature tiles
SB = S // 512             # 512-wide seq blocks
MT = DFF // 128           # dff tiles
REPLICA_GROUPS = [[0, 1, 2, 3], [4, 5, 6, 7]]


def build(repeat=1, qkv_bias=False, ffn_bias=False, no_coll=False):
    nc = bacc.Bacc("TRN2", target_bir_lowering=False, debug=False,
                   num_devices=N_CORES)

    d = {"qkv_bias": qkv_bias, "ffn_bias": ffn_bias, "no_coll": no_coll}
    d["xt"] = nc.dram_tensor("xt", [128, FT, S], BF, kind="ExternalInput")
    d["xres"] = nc.dram_tensor("xres", [128, FT, RC], BF, kind="ExternalInput")
    d["wq"] = nc.dram_tensor("wq", [2, 128, FT, 128], BF, kind="ExternalInput")
    d["wk"] = nc.dram_tensor("wk", [2, 128, FT, 128], BF, kind="ExternalInput")
    d["wv"] = nc.dram_tensor("wv", [128, FT, DC], BF, kind="ExternalInput")
    d["qkvc"] = nc.dram_tensor("qkvc", [6, DC], BF, kind="ExternalInput")
    d["wo"] = nc.dram_tensor("wo", [128, 2, FT, 128], BF, kind="ExternalInput")
    d["w1"] = nc.dram_tensor("w1", [128, MT, FT, 128], BF, kind="ExternalInput")
    d["b1t"] = nc.dram_tensor("b1t", [128, MT], F32, kind="ExternalInput")
    d["w2"] = nc.dram_tensor("w2", [128, MT, FT, 128], BF, kind="ExternalInput")
    d["b2t"] = nc.dram_tensor("b2t", [128, FT], F32, kind="ExternalInput")
    d["mask4"] = nc.dram_tensor("mask4", [4, 128, 512], BF, kind="ExternalInput")
    d["onesp"] = nc.dram_tensor("onesp", [128, 1], BF, kind="ExternalInput")
    d["ones64"] = nc.dram_tensor("ones64", [128, 16, HC, 1], BF, kind="ExternalInput")
    d["out"] = nc.dram_tensor("out", [128, FT, RC], F32, kind="ExternalOutput")

    with tile.TileContext(nc) as tc:
        for _ in range(repeat):
            _emit(nc, tc, d)

    nc.compile()
    return nc


def _emit(nc, tc, d):
    qkv_bias = d["qkv_bias"]
    with (
        tc.tile_pool(name="dram", bufs=1, space="DRAM") as dramp,
        tc.tile_pool(name="const", bufs=1) as cp,
        tc.tile_pool(name="w1p", bufs=1) as w1p,
    ):
        # DRAM bounce buffers for the collective
        wob = dramp.tile([SB, 128, FT, RC], BF, tag="wob")
        hb = dramp.tile([128, FT, RC], BF, tag="hb")
        drs = dramp.tile([1, S], F32, tag="drs")

        # persistent small tensors + weight prefetch (no dependencies)
        onesp = cp.tile([128, 1], BF, tag="onesp")
        b1t = cp.tile([128, MT], F32, tag="b1t")
        b2t = cp.tile([128, FT], F32, tag="b2t")
        rst = cp.tile([128, 16], F32, tag="rst")
        ncst = 6 if qkv_bias else 3
        cst = [cp.tile([1, DC], BF, tag=f"qkvc{i}", name=f"qkvc{i}")
               for i in range(ncst)]
        w1_sb = w1p.tile([128, MT, FT, 128], BF, tag="w1")

        wqs, wks, wvs = cst[0][:], cst[1][:], cst[2][:]
        if qkv_bias:
            bqc, bkc, bvc = cst[3][:], cst[4][:], cst[5][:]

        with tc.tile_pool(name="persa", bufs=1) as pa:
            qt_sb = pa.tile([128, 2, S], BF, tag="qt")
            kt_sb = pa.tile([128, 2, S], BF, tag="kt")
            v_sb = pa.tile([128, 16, HC, 65], BF, tag="v")
            mask4 = pa.tile([128, 4, 512], BF, tag="mask4")
            wo_sb = pa.tile([128, 2, FT, 128], BF, tag="wo")

            # ================= phase 1: LN1 stats + projections ==========
            with (
                tc.tile_pool(name="xpool", bufs=1) as xp,
                tc.tile_pool(name="sq", bufs=1) as sqp,
                tc.tile_pool(name="vtp", bufs=2) as vtp,
                tc.tile_pool(name="stps", bufs=1, space="PSUM") as stps,
                tc.tile_pool(name="prps", bufs=3, space="PSUM") as prps,
            ):
                x_sb = xp.tile([128, FT, S], BF, tag="x")
                wq_sb = xp.tile([128, 2, FT, 128], BF, tag="wq")
                wk_sb = xp.tile([128, 2, FT, 128], BF, tag="wk")
                wv_sb = xp.tile([128, FT, DC], BF, tag="wv")
                nmu_row = xp.tile([1, S], BF, tag="nmu_row")
                std_row = xp.tile([1, S], BF, tag="std_row") if qkv_bias else None
                rs_row = xp.tile([1, S], F32, tag="rs_row")
                a_b = xp.tile([128, S], F32, tag="a_b")

                # x first (critical path), then stage-A weights, then
                # stage-B weights (wo tiny, w1 big) — all on the sync queue.
                for c in range(2):
                    cl = bass.ts(c, 1024)
                    nc.sync.dma_start(out=x_sb[:, :, cl],
                                      in_=d["xt"].ap()[:, :, cl])
                nc.sync.dma_start(out=onesp[:], in_=d["onesp"].ap())
                for i in range(ncst):
                    nc.sync.dma_start(out=cst[i][:],
                                      in_=d["qkvc"].ap()[i:i + 1, :])
                for dd in range(2):
                    nc.sync.dma_start(out=wq_sb[:, dd], in_=d["wq"].ap()[dd])
                    nc.sync.dma_start(out=wk_sb[:, dd], in_=d["wk"].ap()[dd])
                nc.sync.dma_start(out=wv_sb[:], in_=d["wv"].ap())
                nc.sync.dma_start(out=v_sb[:, :, :, 64:65], in_=d["ones64"].ap())
                nc.sync.dma_start(out=mask4[:],
                                  in_=d["mask4"].ap()
                                  .rearrange("j p c -> p j c"))
                nc.sync.dma_start(out=wo_sb[:], in_=d["wo"].ap())
                nc.sync.dma_start(out=b1t[:], in_=d["b1t"].ap())
                nc.sync.dma_start(out=b2t[:], in_=d["b2t"].ap())
                nc.sync.dma_start(out=w1_sb[:], in_=d["w1"].ap())

                def emit_stats(c):
                    cl = bass.ts(c, 1024)
                    # DVE pairwise trees for sum(x) and sum(x^2) over the
                    # 8 feature tiles, then ones-vector matmuls for the
                    # partition reduction (one per 512-wide psum bank).
                    x2 = sqp.tile([128, FT, 1024], BF, tag="x2")
                    t4 = sqp.tile([128, 4, 1024], BF, tag="t4")
                    sums = sqp.tile([128, 2, 1024], BF, tag="sums")
                    nc.vector.tensor_tensor(x2[:], x_sb[:, :, cl], x_sb[:, :, cl],
                                            ALU.mult)
                    nc.vector.tensor_tensor(t4[:], x_sb[:, 0:4, cl],
                                            x_sb[:, 4:8, cl], ALU.add)
                    nc.vector.tensor_tensor(t4[:, 0:2], t4[:, 0:2], t4[:, 2:4],
                                            ALU.add)
                    nc.vector.tensor_tensor(sums[:, 0], t4[:, 0], t4[:, 1],
                                            ALU.add)
                    nc.vector.tensor_tensor(t4[:], x2[:, 0:4], x2[:, 4:8],
                                            ALU.add)
                    nc.vector.tensor_tensor(t4[:, 0:2], t4[:, 0:2], t4[:, 2:4],
                                            ALU.add)
                    nc.vector.tensor_tensor(sums[:, 1], t4[:, 0], t4[:, 1],
                                            ALU.add)
                    # stp[0] = sum(x) over cl, stp2[0] = sum(x^2) over cl
                    stp = stps.tile([1, 2, 512], F32, tag="stp")
                    stp2 = stps.tile([1, 2, 512], F32, tag="stp2")
                    nc.tensor.matmul(stp[:, 0], onesp[:], sums[:, 0, 0:512],
                                     start=True, stop=True)
                    nc.tensor.matmul(stp[:, 1], onesp[:], sums[:, 0, 512:1024],
                                     start=True, stop=True)
                    nc.tensor.matmul(stp2[:, 0], onesp[:], sums[:, 1, 0:512],
                                     start=True, stop=True)
                    nc.tensor.matmul(stp2[:, 1], onesp[:], sums[:, 1, 512:1024],
                                     start=True, stop=True)
                    # row chain on [1, 1024]: nmu = -sum(x)/D ;
                    # var+eps = sum(x2)/D - mu^2 + eps ; rs = (var+eps)^-1/2
                    vt = vtp.tile([1, 2, 1024], F32, tag="vt")
                    flat = stp[0:1, :, :].rearrange("o a b -> o (a b)")
                    flat2 = stp2[0:1, :, :].rearrange("o a b -> o (a b)")
                    nc.vector.tensor_scalar(out=nmu_row[:, cl], in0=flat,
                                            scalar1=-1.0 / D, scalar2=None,
                                            op0=ALU.mult)
                    nc.vector.tensor_tensor(vt[:, 0], nmu_row[:, cl],
                                            nmu_row[:, cl], ALU.mult)
                    nc.vector.tensor_scalar(out=vt[:, 0], in0=vt[:, 0],
                                            scalar1=EPS, scalar2=None,
                                            op0=ALU.subtract)
                    nc.vector.scalar_tensor_tensor(vt[:, 1], flat2, 1.0 / D,
                                                   vt[:, 0], op0=ALU.mult,
                                                   op1=ALU.subtract)
                    nc.scalar.activation(vt[:, 0], vt[:, 1], AF.Ln)
                    nc.scalar.activation(rs_row[:, cl], vt[:, 0], AF.Exp,
                                         scale=-0.5)
                    if qkv_bias:
                        nc.scalar.activation(std_row[:, cl], vt[:, 0], AF.Exp,
                                             scale=0.5)
                    nc.gpsimd.partition_broadcast(a_b[:, cl], rs_row[:, cl])
                    # rs for v-scaling needs a [128, 8] column layout: bounce
                    # through DRAM with a rearranging read.
                    nc.scalar.dma_start(out=drs[:, cl], in_=rs_row[:, cl])
                    nc.gpsimd.dma_start(
                        out=rst[:, 8 * c:8 * c + 8],
                        in_=drs[:, cl].rearrange("o (t p) -> (o p) t", p=128))

                def emit_proj(b):
                    sl = bass.ts(b, 512)
                    for (w_sb, wsum, bc, o_sb) in (
                            (wq_sb, wqs, bqc if qkv_bias else None, qt_sb),
                            (wk_sb, wks, bkc if qkv_bias else None, kt_sb)):
                        for dd in range(2):
                            dsl = bass.ts(dd, 128)
                            ps = prps.tile([128, 512], F32, tag="pp")
                            for f in range(FT):
                                nc.tensor.matmul(ps[:], w_sb[:, dd, f],
                                                 x_sb[:, f, sl],
                                                 start=(f == 0), stop=False)
                            nc.tensor.matmul(ps[:], wsum[0:1, dsl],
                                             nmu_row[:, sl], start=False,
                                             stop=not qkv_bias)
                            if qkv_bias:
                                nc.tensor.matmul(ps[:], bc[0:1, dsl],
                                                 std_row[:, sl],
                                                 start=False, stop=True)
                            nc.vector.tensor_tensor(o_sb[:, dd, sl], ps[:],
                                                    a_b[:, sl], ALU.mult)
                    for i in range(4 * b, 4 * b + 4):
                        rl = bass.ts(i, 128)
                        ps = prps.tile([128, 512], F32, tag="pp")
                        for f in range(FT):
                            nc.tensor.matmul(ps[:, 0:DC], x_sb[:, f, rl],
                                             wv_sb[:, f],
                                             start=(f == 0), stop=False)
                        nc.tensor.matmul(ps[:, 0:DC], nmu_row[:, rl], wvs,
                                         start=False, stop=not qkv_bias)
                        if qkv_bias:
                            nc.tensor.matmul(ps[:, 0:DC], std_row[:, rl], bvc,
                                             start=False, stop=True)
                        nc.vector.tensor_scalar(
                            out=v_sb[:, i, :, 0:64],
                            in0=ps[:, 0:DC].rearrange("p (h e) -> p h e", h=HC),
                            scalar1=rst[:, i:i + 1], scalar2=None,
                            op0=ALU.mult)

                emit_stats(0)
                emit_proj(0)
                emit_stats(1)
                for b in range(1, SB):
                    emit_proj(b)
            # xpool closed

            # ================= phase 2: attention + Wo partials ==========
            with (
                tc.tile_pool(name="atq", bufs=2) as atqp,
                tc.tile_pool(name="wos", bufs=2) as wosp,
                tc.tile_pool(name="exps", bufs=6) as expp,
                tc.tile_pool(name="rcps", bufs=2) as rcpp,
                tc.tile_pool(name="scps", bufs=2, space="PSUM") as scps,
                tc.tile_pool(name="pvps", bufs=4, space="PSUM") as pvps,
            ):
                def emit_wo(attn_t, qi_):
                    # row-parallel Wo partial for seq block qi_ -> bounce
                    wops = wosp.tile([128, FT, 512], BF, tag="wops")
                    for dp in range(FT // 2):
                        ps2 = scps.tile([128, 2, 512], F32, tag="sc")
                        for j in range(2):
                            dd = 2 * dp + j
                            nc.tensor.matmul(ps2[:, j], wo_sb[:, 0, dd],
                                             attn_t[:, 0, :],
                                             start=True, stop=False)
                            nc.tensor.matmul(ps2[:, j], wo_sb[:, 1, dd],
                                             attn_t[:, 1, :],
                                             start=False, stop=True)
                        nc.vector.tensor_copy(out=wops[:, 2 * dp:2 * dp + 2, :],
                                              in_=ps2[:])
                    nc.gpsimd.dma_start(out=wob[qi_], in_=wops[:])

                pending_wo = None
                for qi in range(SB):
                    qsl = bass.ts(qi, 512)
                    attn_qi = atqp.tile([128, 2, 512], BF, tag="attq")
                    nki = 4 * qi + 4
                    pv = {h: pvps.tile([65, 512], F32, tag="pv",
                                       name=f"pv{h}_{qi}")
                          for h in range(2 * HC // 2 * 2)}
                    for hp in range(2):
                        heads = (2 * hp, 2 * hp + 1)
                        for kp in range(nki // 2):
                            ka, kb = 2 * kp, 2 * kp + 1
                            rel0 = 128 * ka - 512 * qi
                            # interleave the two heads: scores for h1 issue
                            # while the exp of h0 runs, so the PE never waits
                            # on the activation engine
                            ex2 = {}
                            for h in heads:
                                hb_ = 64 * (h % 2)
                                hs = slice(hb_, hb_ + 64)
                                sc2 = scps.tile([128, 2, 512], F32, tag="sc",
                                                name=f"sc{h}")
                                nc.tensor.matmul(
                                    sc2[:, 0], kt_sb[hs, hp, bass.ts(ka, 128)],
                                    qt_sb[hs, hp, qsl], start=True, stop=True)
                                nc.tensor.matmul(
                                    sc2[:, 1], kt_sb[hs, hp, bass.ts(kb, 128)],
                                    qt_sb[hs, hp, qsl], start=True, stop=True)
                                e = expp.tile([128, 2, 512], BF, tag="ex",
                                              name=f"ex{h}")
                                nc.scalar.activation(e[:], sc2[:], AF.Exp)
                                if rel0 >= 0:
                                    j = rel0 // 128
                                    mw = rel0 + 256
                                    nc.vector.tensor_tensor(
                                        e[:, :, 0:mw], e[:, :, 0:mw],
                                        mask4[:, j:j + 2, 0:mw], ALU.mult)
                                ex2[h] = e
                            for h in heads:
                                nc.tensor.matmul(pv[h][:], v_sb[:, ka, h, :],
                                                 ex2[h][:, 0], start=(ka == 0),
                                                 stop=False)
                                nc.tensor.matmul(pv[h][:], v_sb[:, kb, h, :],
                                                 ex2[h][:, 1], start=False,
                                                 stop=(kb == nki - 1))
                        if hp == 0 and pending_wo is not None:
                            # emit the previous block's Wo partial here so the
                            # PE has work while this block's softmax-normalize
                            # chain (DVE -> pool -> DVE) completes
                            emit_wo(*pending_wo)
                            pending_wo = None
                    # batched normalize: 4 reciprocals into one row tile, ONE
                    # partition broadcast, then the four per-head multiplies
                    rcp4 = rcpp.tile([1, HC, 512], BF, tag="rcp")
                    rcpb4 = rcpp.tile([64, HC, 512], BF, tag="rcpb")
                    with nc.allow_low_precision(reason="bf16 softmax"):
                        for h in range(HC):
                            nc.vector.reciprocal(rcp4[:, h, :],
                                                 pv[h][64:65, :])
                    nc.gpsimd.partition_broadcast(rcpb4[:], rcp4[:])
                    for h in range(HC):
                        hb_ = 64 * (h % 2)
                        nc.vector.tensor_tensor(
                            attn_qi[hb_:hb_ + 64, h // 2, :],
                            pv[h][0:64, :], rcpb4[:, h, :], ALU.mult)
                    pending_wo = (attn_qi, qi)
                emit_wo(*pending_wo)

                # one bf16 ReduceScatter over the seq axis: rank r receives
                # sum_ranks(partial h) for its own 512 rows
                if d.get("no_coll"):
                    nc.gpsimd.dma_start(out=hb[:], in_=wob[0])
                else:
                    nc.gpsimd.collective_compute(
                        "ReduceScatter", ALU.add, replica_groups=REPLICA_GROUPS,
                        ins=[wob[:].opt()], outs=[hb[:].opt()])
        # persa closed

        # ================= phase 3: residual + LN2 + FFN =================
        with (
            tc.tile_pool(name="pb", bufs=1) as pb,
            tc.tile_pool(name="sq2", bufs=1) as sq2p,
            tc.tile_pool(name="outp", bufs=2) as outp,
        ):
            w2_sb = pb.tile([128, MT, FT, 128], BF, tag="w2")
            xres_sb = pb.tile([128, FT, RC], BF, tag="xres")
            nc.sync.dma_start(out=xres_sb[:], in_=d["xres"].ap())
            nc.sync.dma_start(out=w2_sb[:], in_=d["w2"].ap())
            h_sb = pb.tile([128, FT, RC], BF, tag="h")
            hn_sb = pb.tile([128, FT, RC], BF, tag="hn")
            a_sb = pb.tile([128, MT, RC], BF, tag="a")
            rs2_row = pb.tile([1, RC], BF, tag="rs2_row")
            nmu2_row = pb.tile([1, RC], BF, tag="nmu2_row")
            l2b = pb.tile([1, RC], BF, tag="l2b")
            l2a_b = pb.tile([128, RC], BF, tag="l2a_b")
            l2b_b = pb.tile([128, RC], BF, tag="l2b_b")

            nc.gpsimd.dma_start(out=h_sb[:], in_=hb[:])
            nc.vector.tensor_tensor(h_sb[:], h_sb[:], xres_sb[:], ALU.add)

            # LN2 stats (same tree + ones-matmul + row chain as LN1);
            # hn_sb doubles as the h^2 scratch until hn itself is written
            with tc.tile_pool(name="st2ps", bufs=1, space="PSUM") as st2ps:
                h2 = hn_sb
                t4b = a_sb[:, 0:4, :]
                sums2 = a_sb
                nc.vector.tensor_tensor(h2[:], h_sb[:], h_sb[:], ALU.mult)
                nc.vector.tensor_tensor(t4b, h_sb[:, 0:4], h_sb[:, 4:8],
                                        ALU.add)
                nc.vector.tensor_tensor(t4b[:, 0:2], t4b[:, 0:2], t4b[:, 2:4],
                                        ALU.add)
                nc.vector.tensor_tensor(sums2[:, 4], t4b[:, 0], t4b[:, 1],
                                        ALU.add)
                nc.vector.tensor_tensor(t4b, h2[:, 0:4], h2[:, 4:8], ALU.add)
                nc.vector.tensor_tensor(t4b[:, 0:2], t4b[:, 0:2], t4b[:, 2:4],
                                        ALU.add)
                nc.vector.tensor_tensor(sums2[:, 5], t4b[:, 0], t4b[:, 1],
                                        ALU.add)
                stp2 = st2ps.tile([1, 2, RC], F32, tag="stp2")
                nc.tensor.matmul(stp2[:, 0], onesp[:], sums2[:, 4],
                                 start=True, stop=True)
                nc.tensor.matmul(stp2[:, 1], onesp[:], sums2[:, 5],
                                 start=True, stop=True)
                vt2 = sq2p.tile([1, 2, RC], F32, tag="vt2")
                nc.vector.tensor_scalar(out=nmu2_row[:], in0=stp2[0:1, 0],
                                        scalar1=-1.0 / D, scalar2=None,
                                        op0=ALU.mult)
                nc.vector.tensor_tensor(vt2[:, 0], nmu2_row[:], nmu2_row[:],
                                        ALU.mult)
                nc.vector.tensor_scalar(out=vt2[:, 0], in0=vt2[:, 0],
                                        scalar1=EPS, scalar2=None,
                                        op0=ALU.subtract)
                nc.vector.scalar_tensor_tensor(vt2[:, 1], stp2[0:1, 1], 1.0 / D,
                                               vt2[:, 0], op0=ALU.mult,
                                               op1=ALU.subtract)
                nc.scalar.activation(vt2[:, 0], vt2[:, 1], AF.Ln)
                nc.scalar.activation(rs2_row[:], vt2[:, 0], AF.Exp, scale=-0.5)
                nc.vector.tensor_tensor(l2b[:], nmu2_row[:], rs2_row[:],
                                        ALU.mult)
                nc.gpsimd.partition_broadcast(l2a_b[:], rs2_row[:])
                nc.gpsimd.partition_broadcast(l2b_b[:], l2b[:])

                for f in range(FT):
                    nc.vector.tensor_tensor(hn_sb[:, f], h_sb[:, f], l2a_b[:],
                                            ALU.mult)
                    nc.vector.tensor_add(hn_sb[:, f], hn_sb[:, f], l2b_b[:])

            with (
                tc.tile_pool(name="f1ps", bufs=2, space="PSUM") as f1ps,
                tc.tile_pool(name="f2ps", bufs=2, space="PSUM") as f2ps,
            ):
                for mp in range(MT // 2):
                    ps2 = f1ps.tile([128, 2, RC], F32, tag="f1")
                    for j in range(2):
                        m = 2 * mp + j
                        for f in range(FT):
                            nc.tensor.matmul(ps2[:, j], w1_sb[:, m, f],
                                             hn_sb[:, f],
                                             start=(f == 0), stop=(f == FT - 1))
                    if d["ffn_bias"]:
                        for j in range(2):
                            m = 2 * mp + j
                            nc.scalar.activation(a_sb[:, m], ps2[:, j], AF.Relu,
                                                 bias=b1t[:, m:m + 1])
                    else:
                        nc.scalar.activation(a_sb[:, 2 * mp:2 * mp + 2], ps2[:],
                                             AF.Relu)

                for dp in range(FT // 2):
                    ps2 = f2ps.tile([128, 2, RC], F32, tag="f2")
                    for j in range(2):
                        dd = 2 * dp + j
                        for t in range(MT):
                            nc.tensor.matmul(ps2[:, j], w2_sb[:, t, dd],
                                             a_sb[:, t],
                                             start=(t == 0), stop=(t == MT - 1))
                    ot2 = outp.tile([128, 2, RC], F32, tag="ot")
                    if d["ffn_bias"]:
                        for j in range(2):
                            dd = 2 * dp + j
                            nc.vector.scalar_tensor_tensor(
                                ot2[:, j], ps2[:, j], b2t[:, dd:dd + 1],
                                h_sb[:, dd], op0=ALU.add, op1=ALU.add)
                    else:
                        nc.vector.tensor_tensor(
                            ot2[:], ps2[:], h_sb[:, 2 * dp:2 * dp + 2, :],
                            ALU.add)
                    nc.scalar.dma_start(out=d["out"].ap()[:, 2 * dp:2 * dp + 2],
                                        in_=ot2[:])


# ----------------------------------------------------------------------
# host side
# ----------------------------------------------------------------------

BF_NP = ml_dtypes.bfloat16


def make_in_maps(x, mask, Wq, Wk, Wv, Wo, w1, b1, w2, b2, g1, be1, g2, be2):
    """Build the 8 per-core input maps from the full inputs."""
    f32 = np.float32
    x = np.asarray(x, f32)
    mask = np.asarray(mask)
    Wq, Wk, Wv, Wo = (np.asarray(t, f32) for t in (Wq, Wk, Wv, Wo))
    w1, b1, w2, b2 = (np.asarray(t, f32) for t in (w1, b1, w2, b2))
    g1, be1, g2, be2 = (np.asarray(t, f32) for t in (g1, be1, g2, be2))

    Wq_s = g1[:, None] * Wq / np.sqrt(np.float32(DH))
    Wk_s = g1[:, None] * Wk
    Wv_s = g1[:, None] * Wv
    bq_full = (be1 @ Wq) / np.sqrt(np.float32(DH))
    bk_full = be1 @ Wk
    bv_full = be1 @ Wv
    w1_s = g2[:, None] * w1
    b1_s = b1 + be2 @ w1
    m2d = np.asarray(mask[0, 0], bool)
    mask4 = np.stack([m2d[0:512, 128 * j:128 * j + 128].T.astype(f32)
                      for j in range(4)]).astype(BF_NP)
    onesp = np.ones((128, 1), BF_NP)
    ones64 = np.ones((128, 16, HC, 1), BF_NP)
    b1t = np.ascontiguousarray(b1_s.reshape(MT, 128).T).astype(f32)
    b2t = np.ascontiguousarray(b2.reshape(FT, 128).T).astype(f32)
    w1_p = np.ascontiguousarray(
        w1_s.reshape(FT, 128, MT, 128).transpose(1, 2, 0, 3)).astype(BF_NP)
    w2_p = np.ascontiguousarray(
        w2.reshape(MT, 128, FT, 128).transpose(1, 0, 2, 3)).astype(BF_NP)

    in_maps = []
    for c in range(N_CORES):
        g, r = divmod(c, TP)
        xT = np.ascontiguousarray(x[g].T)                       # [D, S]
        xt = np.ascontiguousarray(
            xT.reshape(FT, 128, S).transpose(1, 0, 2)).astype(BF_NP)
        xres = np.ascontiguousarray(
            xT[:, RC * r:RC * (r + 1)].reshape(FT, 128, RC)
            .transpose(1, 0, 2)).astype(BF_NP)
        sh = slice(DC * r, DC * (r + 1))
        wq_c = np.ascontiguousarray(
            Wq_s[:, sh].reshape(FT, 128, 2, 128).transpose(2, 1, 0, 3)
        ).astype(BF_NP)
        wk_c = np.ascontiguousarray(
            Wk_s[:, sh].reshape(FT, 128, 2, 128).transpose(2, 1, 0, 3)
        ).astype(BF_NP)
        wv_c = np.ascontiguousarray(
            Wv_s[:, sh].reshape(FT, 128, DC).transpose(1, 0, 2)).astype(BF_NP)
        wo_c = np.ascontiguousarray(
            Wo[sh, :].reshape(2, 128, FT, 128).transpose(1, 0, 2, 3)
        ).astype(BF_NP)
        qkvc = np.stack([Wq_s[:, sh].sum(0), Wk_s[:, sh].sum(0),
                         Wv_s[:, sh].sum(0), bq_full[sh], bk_full[sh],
                         bv_full[sh]]).astype(BF_NP)
        in_maps.append({
            "xt": xt, "xres": xres, "wq": wq_c, "wk": wk_c, "wv": wv_c,
            "qkvc": qkvc, "wo": wo_c, "w1": w1_p, "b1t": b1t, "w2": w2_p,
            "b2t": b2t, "mask4": mask4, "onesp": onesp, "ones64": ones64,
        })
    return in_maps


def assemble_output(results):
    """[8 x {out: [128, FT, RC]}] -> [B, S, D] float32."""
    out = np.empty((B, S, D), np.float32)
    for c in range(N_CORES):
        g, r = divmod(c, TP)
        ot = results[c]["out"].transpose(1, 0, 2).reshape(D, RC)  # [D, RC]
        out[g, RC * r:RC * (r + 1), :] = ot.T
    return out


_nc_cache = {}


def get_nc(repeat=1, qkv_bias=False, ffn_bias=False, no_coll=False):
    key = (repeat, qkv_bias, ffn_bias, no_coll)
    if key not in _nc_cache:
        _nc_cache[key] = build(repeat=repeat, qkv_bias=qkv_bias,
                               ffn_bias=ffn_bias, no_coll=no_coll)
    return _nc_cache[key]


def _flags(inputs):
    qkv_bias = bool(np.any(np.asarray(inputs["be1"], np.float32)))
    ffn_bias = bool(np.any(np.asarray(inputs["b1"], np.float32))
                    or np.any(np.asarray(inputs["b2"], np.float32))
                    or np.any(np.asarray(inputs["be2"], np.float32)))
    return qkv_bias, ffn_bias


def kernel(**inputs):
    qkv_bias, ffn_bias = _flags(inputs)
    nc = get_nc(qkv_bias=qkv_bias, ffn_bias=ffn_bias)
    in_maps = make_in_maps(**inputs)
    res = run_bass_kernel_spmd(nc, in_maps, core_ids=list(range(N_CORES)))
    return assemble_output(res.results)


# revision 21
# speedup vs baseline: 1.4027x; 1.4027x over previous
"""Trainium2 Bass kernel: pre-norm decoder block (B=2, S=2048, D=1024, H=16, DFF=4096).

Sharding: 8 cores = 2 data-parallel groups (one per batch) x 4 tensor-parallel
ranks. Attention is head-sharded (4 heads/core, Megatron column-parallel QKV).
Each rank computes its partial Wo contribution (row-parallel Wo) per 512-wide
sequence block as attention for that block completes; a single bf16
ReduceScatter over the sequence axis then hands every rank the fully-reduced
pre-residual h for its own 512 rows. The rest (residual + LN2 + FFN +
residual) runs sequence-sharded with full w1/w2 (no further collectives).

Everything on-chip is bf16 (psum accumulation fp32). Elementwise work is
batched into as few, as wide instructions as possible (paired-psum tiles with
one activation/copy over both halves) since per-instruction overhead
dominates on this target. LayerNorm statistics are computed by DVE
pairwise-reduction trees plus ones-vector matmuls; the per-position LN affine
is folded into the projections via extra contraction rows (LN1) or a
broadcasted scale/shift (LN2).
"""

import numpy as np
import ml_dtypes

import concourse.bass as bass
import concourse.mybir as mybir
import concourse.tile as tile
from concourse import bacc
from concourse.bass_utils import run_bass_kernel_spmd

BF = mybir.dt.bfloat16
F8 = mybir.dt.float8e4
F32 = mybir.dt.float32
AF = mybir.ActivationFunctionType
ALU = mybir.AluOpType

B, S, D, H, DFF = 2, 2048, 1024, 16, 4096
DH = D // H
EPS = 1e-5

N_CORES = 8
TP = 4                    # tensor-parallel ranks per group
HC = H // TP              # heads per core
DC = HC * DH              # head features per core
RC = S // TP              # seq rows per core in stage B
FT = D // 128             # feature tiles
SB = S // 512             # 512-wide seq blocks
MT = DFF // 128           # dff tiles
REPLICA_GROUPS = [[0, 1, 2, 3], [4, 5, 6, 7]]


def build(repeat=1, qkv_bias=False, ffn_bias=False, no_coll=False):
    nc = bacc.Bacc("TRN2", target_bir_lowering=False, debug=False,
                   num_devices=N_CORES)

    d = {"qkv_bias": qkv_bias, "ffn_bias": ffn_bias, "no_coll": no_coll}
    d["xt"] = nc.dram_tensor("xt", [128, FT, S], BF, kind="ExternalInput")
    d["xres"] = nc.dram_tensor("xres", [128, FT, RC], BF, kind="ExternalInput")
    d["wq"] = nc.dram_tensor("wq", [2, 128, FT, 128], BF, kind="ExternalInput")
    d["wk"] = nc.dram_tensor("wk", [2, 128, FT, 128], BF, kind="ExternalInput")
    d["wv"] = nc.dram_tensor("wv", [128, FT, DC], BF, kind="ExternalInput")
    d["qkvc"] = nc.dram_tensor("qkvc", [6, DC], BF, kind="ExternalInput")
    d["wo"] = nc.dram_tensor("wo", [128, 2, FT, 128], BF, kind="ExternalInput")
    d["w1"] = nc.dram_tensor("w1", [128, MT, FT, 128], BF, kind="ExternalInput")
    d["b1t"] = nc.dram_tensor("b1t", [128, MT], F32, kind="ExternalInput")
    d["w2"] = nc.dram_tensor("w2", [128, MT, FT, 128], BF, kind="ExternalInput")
    d["b2t"] = nc.dram_tensor("b2t", [128, FT], F32, kind="ExternalInput")
    d["mask4"] = nc.dram_tensor("mask4", [4, 128, 512], BF, kind="ExternalInput")
    d["onesp"] = nc.dram_tensor("onesp", [128, 1], BF, kind="ExternalInput")
    d["ones64"] = nc.dram_tensor("ones64", [128, 16, HC, 1], BF, kind="ExternalInput")
    d["out"] = nc.dram_tensor("out", [128, FT, RC], F32, kind="ExternalOutput")

    with tile.TileContext(nc) as tc:
        for _ in range(repeat):
            _emit(nc, tc, d)

    nc.compile()
    return nc


def _emit(nc, tc, d):
    qkv_bias = d["qkv_bias"]
    with (
        tc.tile_pool(name="dram", bufs=1, space="DRAM") as dramp,
        tc.tile_pool(name="const", bufs=1) as cp,
        tc.tile_pool(name="w1p", bufs=1) as w1p,
    ):
        # DRAM bounce buffers for the collective
        wob = dramp.tile([SB, 128, FT, RC], BF, tag="wob")
        hb = dramp.tile([128, FT, RC], BF, tag="hb")
        drs = dramp.tile([1, S], F32, tag="drs")

        # persistent small tensors + weight prefetch (no dependencies)
        onesp = cp.tile([128, 1], BF, tag="onesp")
        b1t = cp.tile([128, MT], F32, tag="b1t")
        b2t = cp.tile([128, FT], F32, tag="b2t")
        rst = cp.tile([128, 16], F32, tag="rst")
        ncst = 6 if qkv_bias else 3
        cst = [cp.tile([1, DC], BF, tag=f"qkvc{i}", name=f"qkvc{i}")
               for i in range(ncst)]
        w1_sb = w1p.tile([128, MT, FT, 128], BF, tag="w1")

        wqs, wks, wvs = cst[0][:], cst[1][:], cst[2][:]
        if qkv_bias:
            bqc, bkc, bvc = cst[3][:], cst[4][:], cst[5][:]

        with tc.tile_pool(name="persa", bufs=1) as pa:
            qt_sb = pa.tile([128, 2, S], BF, tag="qt")
            kt_sb = pa.tile([128, 2, S], BF, tag="kt")
            v_sb = pa.tile([128, 16, HC, 65], BF, tag="v")
            mask4 = pa.tile([128, 4, 512], BF, tag="mask4")
            wo_sb = pa.tile([128, 2, FT, 128], BF, tag="wo")

            # ================= phase 1: LN1 stats + projections ==========
            with (
                tc.tile_pool(name="xpool", bufs=1) as xp,
                tc.tile_pool(name="sq", bufs=1) as sqp,
                tc.tile_pool(name="vtp", bufs=2) as vtp,
                tc.tile_pool(name="stps", bufs=1, space="PSUM") as stps,
                tc.tile_pool(name="prps", bufs=3, space="PSUM") as prps,
            ):
                x_sb = xp.tile([128, FT, S], BF, tag="x")
                wq_sb = xp.tile([128, 2, FT, 128], BF, tag="wq")
                wk_sb = xp.tile([128, 2, FT, 128], BF, tag="wk")
                wv_sb = xp.tile([128, FT, DC], BF, tag="wv")
                nmu_row = xp.tile([1, S], BF, tag="nmu_row")
                std_row = xp.tile([1, S], BF, tag="std_row") if qkv_bias else None
                rs_row = xp.tile([1, S], F32, tag="rs_row")
                a_b = xp.tile([128, S], F32, tag="a_b")

                # x first (critical path), then stage-A weights, then
                # stage-B weights (wo tiny, w1 big) — all on the sync queue.
                for c in range(2):
                    cl = bass.ts(c, 1024)
                    nc.sync.dma_start(out=x_sb[:, :, cl],
                                      in_=d["xt"].ap()[:, :, cl])
                nc.sync.dma_start(out=onesp[:], in_=d["onesp"].ap())
                for i in range(ncst):
                    nc.sync.dma_start(out=cst[i][:],
                                      in_=d["qkvc"].ap()[i:i + 1, :])
                for dd in range(2):
                    nc.sync.dma_start(out=wq_sb[:, dd], in_=d["wq"].ap()[dd])
                    nc.sync.dma_start(out=wk_sb[:, dd], in_=d["wk"].ap()[dd])
                nc.sync.dma_start(out=wv_sb[:], in_=d["wv"].ap())
                nc.sync.dma_start(out=v_sb[:, :, :, 64:65], in_=d["ones64"].ap())
                nc.sync.dma_start(out=mask4[:],
                                  in_=d["mask4"].ap()
                                  .rearrange("j p c -> p j c"))
                nc.sync.dma_start(out=wo_sb[:], in_=d["wo"].ap())
                nc.sync.dma_start(out=b1t[:], in_=d["b1t"].ap())
                nc.sync.dma_start(out=b2t[:], in_=d["b2t"].ap())
                nc.sync.dma_start(out=w1_sb[:], in_=d["w1"].ap())

                def emit_stats(c):
                    cl = bass.ts(c, 1024)
                    # DVE pairwise trees for sum(x) and sum(x^2) over the
                    # 8 feature tiles, then ones-vector matmuls for the
                    # partition reduction (one per 512-wide psum bank).
                    x2 = sqp.tile([128, FT, 1024], BF, tag="x2")
                    t4 = sqp.tile([128, 4, 1024], BF, tag="t4")
                    sums = sqp.tile([128, 2, 1024], BF, tag="sums")
                    nc.vector.tensor_tensor(x2[:], x_sb[:, :, cl], x_sb[:, :, cl],
                                            ALU.mult)
                    nc.vector.tensor_tensor(t4[:], x_sb[:, 0:4, cl],
                                            x_sb[:, 4:8, cl], ALU.add)
                    nc.vector.tensor_tensor(t4[:, 0:2], t4[:, 0:2], t4[:, 2:4],
                                            ALU.add)
                    nc.vector.tensor_tensor(sums[:, 0], t4[:, 0], t4[:, 1],
                                            ALU.add)
                    nc.vector.tensor_tensor(t4[:], x2[:, 0:4], x2[:, 4:8],
                                            ALU.add)
                    nc.vector.tensor_tensor(t4[:, 0:2], t4[:, 0:2], t4[:, 2:4],
                                            ALU.add)
                    nc.vector.tensor_tensor(sums[:, 1], t4[:, 0], t4[:, 1],
                                            ALU.add)
                    # stp[0] = sum(x) over cl, stp2[0] = sum(x^2) over cl
                    stp = stps.tile([1, 2, 512], F32, tag="stp")
                    stp2 = stps.tile([1, 2, 512], F32, tag="stp2")
                    nc.tensor.matmul(stp[:, 0], onesp[:], sums[:, 0, 0:512],
                                     start=True, stop=True)
                    nc.tensor.matmul(stp[:, 1], onesp[:], sums[:, 0, 512:1024],
                                     start=True, stop=True)
                    nc.tensor.matmul(stp2[:, 0], onesp[:], sums[:, 1, 0:512],
                                     start=True, stop=True)
                    nc.tensor.matmul(stp2[:, 1], onesp[:], sums[:, 1, 512:1024],
                                     start=True, stop=True)
                    # row chain on [1, 1024]: nmu = -sum(x)/D ;
                    # var+eps = sum(x2)/D - mu^2 + eps ; rs = (var+eps)^-1/2
                    vt = vtp.tile([1, 2, 1024], F32, tag="vt")
                    flat = stp[0:1, :, :].rearrange("o a b -> o (a b)")
                    flat2 = stp2[0:1, :, :].rearrange("o a b -> o (a b)")
                    nc.vector.tensor_scalar(out=nmu_row[:, cl], in0=flat,
                                            scalar1=-1.0 / D, scalar2=None,
                                            op0=ALU.mult)
                    nc.vector.tensor_tensor(vt[:, 0], nmu_row[:, cl],
                                            nmu_row[:, cl], ALU.mult)
                    nc.vector.tensor_scalar(out=vt[:, 0], in0=vt[:, 0],
                                            scalar1=EPS, scalar2=None,
                                            op0=ALU.subtract)
                    nc.vector.scalar_tensor_tensor(vt[:, 1], flat2, 1.0 / D,
                                                   vt[:, 0], op0=ALU.mult,
                                                   op1=ALU.subtract)
                    nc.scalar.activation(vt[:, 0], vt[:, 1], AF.Ln)
                    nc.scalar.activation(rs_row[:, cl], vt[:, 0], AF.Exp,
                                         scale=-0.5)
                    if qkv_bias:
                        nc.scalar.activation(std_row[:, cl], vt[:, 0], AF.Exp,
                                             scale=0.5)
                    nc.gpsimd.partition_broadcast(a_b[:, cl], rs_row[:, cl])
                    # rs for v-scaling needs a [128, 8] column layout: bounce
                    # through DRAM with a rearranging read.
                    nc.scalar.dma_start(out=drs[:, cl], in_=rs_row[:, cl])
                    nc.gpsimd.dma_start(
                        out=rst[:, 8 * c:8 * c + 8],
                        in_=drs[:, cl].rearrange("o (t p) -> (o p) t", p=128))

                def emit_proj(b):
                    sl = bass.ts(b, 512)
                    for (w_sb, wsum, bc, o_sb) in (
                            (wq_sb, wqs, bqc if qkv_bias else None, qt_sb),
                            (wk_sb, wks, bkc if qkv_bias else None, kt_sb)):
                        for dd in range(2):
                            dsl = bass.ts(dd, 128)
                            ps = prps.tile([128, 512], F32, tag="pp")
                            for f in range(FT):
                                nc.tensor.matmul(ps[:], w_sb[:, dd, f],
                                                 x_sb[:, f, sl],
                                                 start=(f == 0), stop=False)
                            nc.tensor.matmul(ps[:], wsum[0:1, dsl],
                                             nmu_row[:, sl], start=False,
                                             stop=not qkv_bias)
                            if qkv_bias:
                                nc.tensor.matmul(ps[:], bc[0:1, dsl],
                                                 std_row[:, sl],
                                                 start=False, stop=True)
                            nc.vector.tensor_tensor(o_sb[:, dd, sl], ps[:],
                                                    a_b[:, sl], ALU.mult)
                    for i in range(4 * b, 4 * b + 4):
                        rl = bass.ts(i, 128)
                        ps = prps.tile([128, 512], F32, tag="pp")
                        for f in range(FT):
                            nc.tensor.matmul(ps[:, 0:DC], x_sb[:, f, rl],
                                             wv_sb[:, f],
                                             start=(f == 0), stop=False)
                        nc.tensor.matmul(ps[:, 0:DC], nmu_row[:, rl], wvs,
                                         start=False, stop=not qkv_bias)
                        if qkv_bias:
                            nc.tensor.matmul(ps[:, 0:DC], std_row[:, rl], bvc,
                                             start=False, stop=True)
                        nc.vector.tensor_scalar(
                            out=v_sb[:, i, :, 0:64],
                            in0=ps[:, 0:DC].rearrange("p (h e) -> p h e", h=HC),
                            scalar1=rst[:, i:i + 1], scalar2=None,
                            op0=ALU.mult)

                emit_stats(0)
                emit_proj(0)
                emit_stats(1)
                for b in range(1, SB):
                    emit_proj(b)
            # xpool closed

            # ================= phase 2: attention + Wo partials ==========
            with (
                tc.tile_pool(name="atq", bufs=2) as atqp,
                tc.tile_pool(name="wos", bufs=2) as wosp,
                tc.tile_pool(name="exps", bufs=8) as expp,
                tc.tile_pool(name="rcps", bufs=2) as rcpp,
                tc.tile_pool(name="scps", bufs=2, space="PSUM") as scps,
                tc.tile_pool(name="pvps", bufs=4, space="PSUM") as pvps,
            ):
                def emit_wo(attn_t, qi_):
                    # row-parallel Wo partial for seq block qi_ -> bounce
                    wops = wosp.tile([128, FT, 512], BF, tag="wops")
                    for dp in range(FT // 2):
                        ps2 = scps.tile([128, 2, 512], F32, tag="sc")
                        for j in range(2):
                            dd = 2 * dp + j
                            nc.tensor.matmul(ps2[:, j], wo_sb[:, 0, dd],
                                             attn_t[:, 0, :],
                                             start=True, stop=False)
                            nc.tensor.matmul(ps2[:, j], wo_sb[:, 1, dd],
                                             attn_t[:, 1, :],
                                             start=False, stop=True)
                        nc.vector.tensor_copy(out=wops[:, 2 * dp:2 * dp + 2, :],
                                              in_=ps2[:])
                    nc.gpsimd.dma_start(out=wob[qi_], in_=wops[:])

                pending_wo = None
                for qi in range(SB):
                    qsl = bass.ts(qi, 512)
                    attn_qi = atqp.tile([128, 2, 512], BF, tag="attq")
                    nki = 4 * qi + 4
                    pv = {h: pvps.tile([65, 512], F32, tag="pv",
                                       name=f"pv{h}_{qi}")
                          for h in range(2 * HC // 2 * 2)}
                    for hp in range(2):
                        heads = (2 * hp, 2 * hp + 1)
                        for kp in range(nki // 2):
                            ka, kb = 2 * kp, 2 * kp + 1
                            rel0 = 128 * ka - 512 * qi
                            # interleave the two heads: scores for h1 issue
                            # while the exp of h0 runs, so the PE never waits
                            # on the activation engine
                            ex2 = {}
                            for h in heads:
                                hb_ = 64 * (h % 2)
                                hs = slice(hb_, hb_ + 64)
                                sc2 = scps.tile([128, 2, 512], F32, tag="sc",
                                                name=f"sc{h}")
                                nc.tensor.matmul(
                                    sc2[:, 0], kt_sb[hs, hp, bass.ts(ka, 128)],
                                    qt_sb[hs, hp, qsl], start=True, stop=True)
                                nc.tensor.matmul(
                                    sc2[:, 1], kt_sb[hs, hp, bass.ts(kb, 128)],
                                    qt_sb[hs, hp, qsl], start=True, stop=True)
                                e = expp.tile([128, 2, 512], BF, tag="ex",
                                              name=f"ex{h}")
                                nc.scalar.activation(e[:], sc2[:], AF.Exp)
                                if rel0 >= 0:
                                    j = rel0 // 128
                                    mw = rel0 + 256
                                    nc.vector.tensor_tensor(
                                        e[:, :, 0:mw], e[:, :, 0:mw],
                                        mask4[:, j:j + 2, 0:mw], ALU.mult)
                                ex2[h] = e
                            for h in heads:
                                nc.tensor.matmul(pv[h][:], v_sb[:, ka, h, :],
                                                 ex2[h][:, 0], start=(ka == 0),
                                                 stop=False)
                                nc.tensor.matmul(pv[h][:], v_sb[:, kb, h, :],
                                                 ex2[h][:, 1], start=False,
                                                 stop=(kb == nki - 1))
                        if hp == 0 and pending_wo is not None:
                            # emit the previous block's Wo partial here so the
                            # PE has work while this block's softmax-normalize
                            # chain (DVE -> pool -> DVE) completes
                            emit_wo(*pending_wo)
                            pending_wo = None
                    # batched normalize: 4 reciprocals into one row tile, ONE
                    # partition broadcast, then the four per-head multiplies
                    rcp4 = rcpp.tile([1, HC, 512], BF, tag="rcp")
                    rcpb4 = rcpp.tile([64, HC, 512], BF, tag="rcpb")
                    with nc.allow_low_precision(reason="bf16 softmax"):
                        for h in range(HC):
                            nc.vector.reciprocal(rcp4[:, h, :],
                                                 pv[h][64:65, :])
                    nc.gpsimd.partition_broadcast(rcpb4[:], rcp4[:])
                    for h in range(HC):
                        hb_ = 64 * (h % 2)
                        nc.vector.tensor_tensor(
                            attn_qi[hb_:hb_ + 64, h // 2, :],
                            pv[h][0:64, :], rcpb4[:, h, :], ALU.mult)
                    pending_wo = (attn_qi, qi)
                emit_wo(*pending_wo)

                # one bf16 ReduceScatter over the seq axis: rank r receives
                # sum_ranks(partial h) for its own 512 rows
                if d.get("no_coll"):
                    nc.gpsimd.dma_start(out=hb[:], in_=wob[0])
                else:
                    nc.gpsimd.collective_compute(
                        "ReduceScatter", ALU.add, replica_groups=REPLICA_GROUPS,
                        ins=[wob[:].opt()], outs=[hb[:].opt()])
        # persa closed

        # ================= phase 3: residual + LN2 + FFN =================
        with (
            tc.tile_pool(name="pb", bufs=1) as pb,
            tc.tile_pool(name="sq2", bufs=1) as sq2p,
            tc.tile_pool(name="outp", bufs=2) as outp,
        ):
            w2_sb = pb.tile([128, MT, FT, 128], BF, tag="w2")
            xres_sb = pb.tile([128, FT, RC], BF, tag="xres")
            nc.sync.dma_start(out=xres_sb[:], in_=d["xres"].ap())
            nc.sync.dma_start(out=w2_sb[:], in_=d["w2"].ap())
            h_sb = pb.tile([128, FT, RC], BF, tag="h")
            hn_sb = pb.tile([128, FT, RC], BF, tag="hn")
            a_sb = pb.tile([128, MT, RC], BF, tag="a")
            rs2_row = pb.tile([1, RC], BF, tag="rs2_row")
            nmu2_row = pb.tile([1, RC], BF, tag="nmu2_row")
            l2b = pb.tile([1, RC], BF, tag="l2b")
            l2a_b = pb.tile([128, RC], BF, tag="l2a_b")
            l2b_b = pb.tile([128, RC], BF, tag="l2b_b")

            nc.gpsimd.dma_start(out=h_sb[:], in_=hb[:])
            nc.vector.tensor_tensor(h_sb[:], h_sb[:], xres_sb[:], ALU.add)

            # LN2 stats (same tree + ones-matmul + row chain as LN1);
            # hn_sb doubles as the h^2 scratch until hn itself is written
            with tc.tile_pool(name="st2ps", bufs=1, space="PSUM") as st2ps:
                h2 = hn_sb
                t4b = a_sb[:, 0:4, :]
                sums2 = a_sb
                nc.vector.tensor_tensor(h2[:], h_sb[:], h_sb[:], ALU.mult)
                nc.vector.tensor_tensor(t4b, h_sb[:, 0:4], h_sb[:, 4:8],
                                        ALU.add)
                nc.vector.tensor_tensor(t4b[:, 0:2], t4b[:, 0:2], t4b[:, 2:4],
                                        ALU.add)
                nc.vector.tensor_tensor(sums2[:, 4], t4b[:, 0], t4b[:, 1],
                                        ALU.add)
                nc.vector.tensor_tensor(t4b, h2[:, 0:4], h2[:, 4:8], ALU.add)
                nc.vector.tensor_tensor(t4b[:, 0:2], t4b[:, 0:2], t4b[:, 2:4],
                                        ALU.add)
                nc.vector.tensor_tensor(sums2[:, 5], t4b[:, 0], t4b[:, 1],
                                        ALU.add)
                stp2 = st2ps.tile([1, 2, RC], F32, tag="stp2")
                nc.tensor.matmul(stp2[:, 0], onesp[:], sums2[:, 4],
                                 start=True, stop=True)
                nc.tensor.matmul(stp2[:, 1], onesp[:], sums2[:, 5],
                                 start=True, stop=True)
                vt2 = sq2p.tile([1, 2, RC], F32, tag="vt2")
                nc.vector.tensor_scalar(out=nmu2_row[:], in0=stp2[0:1, 0],
                                        scalar1=-1.0 / D, scalar2=None,
                                        op0=ALU.mult)
                nc.vector.tensor_tensor(vt2[:, 0], nmu2_row[:], nmu2_row[:],
                                        ALU.mult)
                nc.vector.tensor_scalar(out=vt2[:, 0], in0=vt2[:, 0],
                                        scalar1=EPS, scalar2=None,
                                        op0=ALU.subtract)
                nc.vector.scalar_tensor_tensor(vt2[:, 1], stp2[0:1, 1], 1.0 / D,
                                               vt2[:, 0], op0=ALU.mult,
                                               op1=ALU.subtract)
                nc.scalar.activation(vt2[:, 0], vt2[:, 1], AF.Ln)
                nc.scalar.activation(rs2_row[:], vt2[:, 0], AF.Exp, scale=-0.5)
                nc.vector.tensor_tensor(l2b[:], nmu2_row[:], rs2_row[:],
                                        ALU.mult)
                nc.gpsimd.partition_broadcast(l2a_b[:], rs2_row[:])
                nc.gpsimd.partition_broadcast(l2b_b[:], l2b[:])

                for f in range(FT):
                    nc.vector.tensor_tensor(hn_sb[:, f], h_sb[:, f], l2a_b[:],
                                            ALU.mult)
                    nc.vector.tensor_add(hn_sb[:, f], hn_sb[:, f], l2b_b[:])

            with (
                tc.tile_pool(name="f1ps", bufs=2, space="PSUM") as f1ps,
                tc.tile_pool(name="f2ps", bufs=2, space="PSUM") as f2ps,
            ):
                for mp in range(MT // 2):
                    ps2 = f1ps.tile([128, 2, RC], F32, tag="f1")
                    for j in range(2):
                        m = 2 * mp + j
                        for f in range(FT):
                            nc.tensor.matmul(ps2[:, j], w1_sb[:, m, f],
                                             hn_sb[:, f],
                                             start=(f == 0), stop=(f == FT - 1))
                    if d["ffn_bias"]:
                        for j in range(2):
                            m = 2 * mp + j
                            nc.scalar.activation(a_sb[:, m], ps2[:, j], AF.Relu,
                                                 bias=b1t[:, m:m + 1])
                    else:
                        nc.scalar.activation(a_sb[:, 2 * mp:2 * mp + 2], ps2[:],
                                             AF.Relu)

                for dp in range(FT // 2):
                    ps2 = f2ps.tile([128, 2, RC], F32, tag="f2")
                    for j in range(2):
                        dd = 2 * dp + j
                        for t in range(MT):
                            nc.tensor.matmul(ps2[:, j], w2_sb[:, t, dd],
                                             a_sb[:, t],
                                             start=(t == 0), stop=(t == MT - 1))
                    ot2 = outp.tile([128, 2, RC], F32, tag="ot")
                    if d["ffn_bias"]:
                        for j in range(2):
                            dd = 2 * dp + j
                            nc.vector.scalar_tensor_tensor(
                                ot2[:, j], ps2[:, j], b2t[:, dd:dd + 1],
                                h_sb[:, dd], op0=ALU.add, op1=ALU.add)
                    else:
                        nc.vector.tensor_tensor(
                            ot2[:], ps2[:], h_sb[:, 2 * dp:2 * dp + 2, :],
                            ALU.add)
                    nc.scalar.dma_start(out=d["out"].ap()[:, 2 * dp:2 * dp + 2],
                                        in_=ot2[:])


# ----------------------------------------------------------------------
# host side
# ----------------------------------------------------------------------

BF_NP = ml_dtypes.bfloat16


def make_in_maps(x, mask, Wq, Wk, Wv, Wo, w1, b1, w2, b2, g1, be1, g2, be2):
    """Build the 8 per-core input maps from the full inputs."""
    f32 = np.float32
    x = np.asarray(x, f32)
    mask = np.asarray(mask)
    Wq, Wk, Wv, Wo = (np.asarray(t, f32) for t in (Wq, Wk, Wv, Wo))
    w1, b1, w2, b2 = (np.asarray(t, f32) for t in (w1, b1, w2, b2))
    g1, be1, g2, be2 = (np.asarray(t, f32) for t in (g1, be1, g2, be2))

    Wq_s = g1[:, None] * Wq / np.sqrt(np.float32(DH))
    Wk_s = g1[:, None] * Wk
    Wv_s = g1[:, None] * Wv
    bq_full = (be1 @ Wq) / np.sqrt(np.float32(DH))
    bk_full = be1 @ Wk
    bv_full = be1 @ Wv
    w1_s = g2[:, None] * w1
    b1_s = b1 + be2 @ w1
    m2d = np.asarray(mask[0, 0], bool)
    mask4 = np.stack([m2d[0:512, 128 * j:128 * j + 128].T.astype(f32)
                      for j in range(4)]).astype(BF_NP)
    onesp = np.ones((128, 1), BF_NP)
    ones64 = np.ones((128, 16, HC, 1), BF_NP)
    b1t = np.ascontiguousarray(b1_s.reshape(MT, 128).T).astype(f32)
    b2t = np.ascontiguousarray(b2.reshape(FT, 128).T).astype(f32)
    w1_p = np.ascontiguousarray(
        w1_s.reshape(FT, 128, MT, 128).transpose(1, 2, 0, 3)).astype(BF_NP)
    w2_p = np.ascontiguousarray(
        w2.reshape(MT, 128, FT, 128).transpose(1, 0, 2, 3)).astype(BF_NP)

    in_maps = []
    for c in range(N_CORES):
        g, r = divmod(c, TP)
        xT = np.ascontiguousarray(x[g].T)                       # [D, S]
        xt = np.ascontiguousarray(
            xT.reshape(FT, 128, S).transpose(1, 0, 2)).astype(BF_NP)
        xres = np.ascontiguousarray(
            xT[:, RC * r:RC * (r + 1)].reshape(FT, 128, RC)
            .transpose(1, 0, 2)).astype(BF_NP)
        sh = slice(DC * r, DC * (r + 1))
        wq_c = np.ascontiguousarray(
            Wq_s[:, sh].reshape(FT, 128, 2, 128).transpose(2, 1, 0, 3)
        ).astype(BF_NP)
        wk_c = np.ascontiguousarray(
            Wk_s[:, sh].reshape(FT, 128, 2, 128).transpose(2, 1, 0, 3)
        ).astype(BF_NP)
        wv_c = np.ascontiguousarray(
            Wv_s[:, sh].reshape(FT, 128, DC).transpose(1, 0, 2)).astype(BF_NP)
        wo_c = np.ascontiguousarray(
            Wo[sh, :].reshape(2, 128, FT, 128).transpose(1, 0, 2, 3)
        ).astype(BF_NP)
        qkvc = np.stack([Wq_s[:, sh].sum(0), Wk_s[:, sh].sum(0),
                         Wv_s[:, sh].sum(0), bq_full[sh], bk_full[sh],
                         bv_full[sh]]).astype(BF_NP)
        in_maps.append({
            "xt": xt, "xres": xres, "wq": wq_c, "wk": wk_c, "wv": wv_c,
            "qkvc": qkvc, "wo": wo_c, "w1": w1_p, "b1t": b1t, "w2": w2_p,
            "b2t": b2t, "mask4": mask4, "onesp": onesp, "ones64": ones64,
        })
    return in_maps


def assemble_output(results):
    """[8 x {out: [128, FT, RC]}] -> [B, S, D] float32."""
    out = np.empty((B, S, D), np.float32)
    for c in range(N_CORES):
        g, r = divmod(c, TP)
        ot = results[c]["out"].transpose(1, 0, 2).reshape(D, RC)  # [D, RC]
        out[g, RC * r:RC * (r + 1), :] = ot.T
    return out


_nc_cache = {}


def get_nc(repeat=1, qkv_bias=False, ffn_bias=False, no_coll=False):
    key = (repeat, qkv_bias, ffn_bias, no_coll)
    if key not in _nc_cache:
        _nc_cache[key] = build(repeat=repeat, qkv_bias=qkv_bias,
                               ffn_bias=ffn_bias, no_coll=no_coll)
    return _nc_cache[key]


def _flags(inputs):
    qkv_bias = bool(np.any(np.asarray(inputs["be1"], np.float32)))
    ffn_bias = bool(np.any(np.asarray(inputs["b1"], np.float32))
                    or np.any(np.asarray(inputs["b2"], np.float32))
                    or np.any(np.asarray(inputs["be2"], np.float32)))
    return qkv_bias, ffn_bias


def kernel(**inputs):
    qkv_bias, ffn_bias = _flags(inputs)
    nc = get_nc(qkv_bias=qkv_bias, ffn_bias=ffn_bias)
    in_maps = make_in_maps(**inputs)
    res = run_bass_kernel_spmd(nc, in_maps, core_ids=list(range(N_CORES)))
    return assemble_output(res.results)
